# revision 4
# baseline (speedup 1.0000x reference)
"""AttnGraphPooling Trainium2 kernel v3 (8 NeuronCores, SPMD).

v3 vs v2: bigger batching + engine rebalance.
- m=6 lane packing (lane holds up to 6 nodes of one graph across the 6
  chunks of a group). Groups of 6 chunks, processed as two 3-chunk
  windows that map to one 3-bank PSUM tile each (2 windows in flight +
  seg banks = 8 banks exactly).
- ONE fT DMA per group ([128, 1536] fp16, both D-halves concatenated)
  and one oh DMA per 4 groups -> ~1.3 DMA triggers/group (sync engine
  relief; triggers cost ~620ns each on the sync queue).
- ACT exp and DVE mul run once per 3-chunk window (amortizes the
  per-instruction PSUM/SBUF access bubbles: ~185ns ACT, ~125ns DVE).
- Reduction 6->1 per group: R1 = st[0:3]+st[3:6] on DVE (all-fp16
  packed, 2x path), R2/R3 on GpSimd (Pool) to keep DVE under the PE
  cadence. One seg matmul per group accumulates oh^T @ acc into the
  block's PSUM bank (deferred 2 groups so PE never waits on the tree).

Padding slots use f_pad = Wk^{-1} @ (-25*ones): their attn row is ~-25
so exp -> 0 in fp16, killing their contribution (no masks). Pad lanes
additionally get an all-zero one-hot row. Epilogue is ACT-table-free
(Newton rsqrt) so the Exp table never swaps.
"""

import os as _os

import numpy as np

import concourse.bass as bass
import concourse.mybir as mybir
import concourse.tile as tile
from concourse.bass_utils import run_bass_kernel_spmd

N_CORES = 8
D = 256
GBLK = 128
BPC = 4  # blocks (of 128 graphs) per core
PACK = int(_os.environ.get("BASS_KERNEL_PACK", "6"))
SEG_DELAY = int(_os.environ.get("BASS_KERNEL_SEGDELAY", "4"))
OH_SUPER = 4  # groups per oh DMA

EPS_SOFTMAX = 1e-7
EPS_LN = 1e-5
PAD_ATT = -25.0

LAST_EXEC_TIME_NS = None
LAST_RESULTS = None
_nc_cache = {}


def _split_waits(nc, maxw=1):
    cnt = 0
    for f in nc.m.functions:
        for bb in f.blocks:
            newinsts = []
            for inst in bb.instructions:
                si = getattr(inst, "sync_info", None)
                if si is not None and si.on_wait and len(si.on_wait) > maxw:
                    waits = list(si.on_wait)
                    excess = waits[:-maxw]
                    si.on_wait = waits[-maxw:]
                    for i in range(0, len(excess), maxw):
                        nop = mybir.InstNoOp(
                            name=f"Wsplit-{cnt}",
                            engine=inst.engine,
                            bass_nofuse=True,
                            sync_info=mybir.SyncInfo(
                                on_wait=excess[i : i + maxw], on_update=[]
                            ),
                        )
                        cnt += 1
                        newinsts.append(nop)
                newinsts.append(inst)
            bb.instructions = newinsts
    return cnt


def _build_nc3(gpbs, m):
    """gpbs: tuple of groups-per-block for the BPC block slots (same on
    every core); m: lane depth (chunks per group). m must be 6."""
    from contextlib import ExitStack

    R = mybir.dt.float16
    F32 = mybir.dt.float32
    TG = sum(gpbs)
    assert m == 6
    GW = m * 128  # node cols per group (one half)
    FTW = 2 * GW  # fT cols per group (both halves)

    nc = bass.Bass()
    fT_d = nc.dram_tensor("fT", [128, TG * FTW], R, kind="ExternalInput")
    oh_d = nc.dram_tensor("oh", [128, TG * GBLK], R, kind="ExternalInput")
    wcat_d = nc.dram_tensor("wcat", [2, 128, 2 * D], R, kind="ExternalInput")
    vb_d = nc.dram_tensor("vbrep", [128, D], F32, kind="ExternalInput")
    epsd_d = nc.dram_tensor("epsrep", [128, D], F32, kind="ExternalInput")
    gm_d = nc.dram_tensor("gammarep", [128, D], F32, kind="ExternalInput")
    bt_d = nc.dram_tensor("betarep", [128, D], F32, kind="ExternalInput")
    y_d = nc.dram_tensor("y", [BPC * GBLK, D], F32, kind="ExternalOutput")

    with tile.TileContext(nc) as tc, ExitStack() as ctx:
        const = ctx.enter_context(tc.tile_pool(name="const", bufs=1))
        ftp = ctx.enter_context(tc.tile_pool(name="ft", bufs=3))
        ohp = ctx.enter_context(tc.tile_pool(name="oh", bufs=3))
        stp = ctx.enter_context(tc.tile_pool(name="st", bufs=3))
        rp = ctx.enter_context(tc.tile_pool(name="r", bufs=3))
        rrp = ctx.enter_context(tc.tile_pool(name="rr", bufs=3))
        accp = ctx.enter_context(tc.tile_pool(name="acc", bufs=6))
        epi = ctx.enter_context(tc.tile_pool(name="epi", bufs=2))
        pp_pool = ctx.enter_context(tc.tile_pool(name="pp", bufs=2, space="PSUM"))
        seg_pool = ctx.enter_context(tc.tile_pool(name="seg", bufs=2, space="PSUM"))

        # first fT tile first so the PE starts ASAP
        ft_first = ftp.tile([128, FTW], R, tag="ft")
        nc.sync.dma_start(ft_first[:], fT_d[:, 0:FTW])
        w0 = const.tile([128, 2 * D], R, tag="w0")
        nc.sync.dma_start(w0[:], wcat_d[0])
        w1 = const.tile([128, 2 * D], R, tag="w1")
        nc.sync.dma_start(w1[:], wcat_d[1])
        oh_first = ohp.tile([128, OH_SUPER * GBLK], R, tag="oh")
        nc.sync.dma_start(oh_first[:], oh_d[:, 0 : OH_SUPER * GBLK])
        vb = const.tile([128, D], F32, tag="vb")
        nc.sync.dma_start(vb[:], vb_d[:])
        epsd = const.tile([128, D], F32, tag="epsd")
        nc.sync.dma_start(epsd[:], epsd_d[:])
        gm = const.tile([128, D], F32, tag="gm")
        nc.sync.dma_start(gm[:], gm_d[:])
        bt = const.tile([128, D], F32, tag="bt")
        nc.sync.dma_start(bt[:], bt_d[:])
        epsln = const.tile([128, 1], F32, tag="epsln")
        nc.gpsimd.memset(epsln[:], float(EPS_LN))
        magic = const.tile([128, 1], mybir.dt.uint32, tag="magic")
        nc.gpsimd.memset(magic[:], 0x5F3759DF)

        warm = const.tile([128, 1], F32, tag="warm")
        warm2 = const.tile([128, 1], F32, tag="warm2")
        nc.gpsimd.memset(warm[:], 1.0)
        nc.scalar.activation(warm2[:], warm[:], mybir.ActivationFunctionType.Exp)

        seg_tiles = {}
        pending_seg = []

        def emit_seg(n):
            # emit up to n queued seg matmuls (oldest first)
            for _ in range(min(n, len(pending_seg))):
                blk, oht, acct, start, stop = pending_seg.pop(0)
                nc.tensor.matmul(
                    seg_tiles[blk][:],
                    oht,
                    acct[:],
                    start=start,
                    stop=stop,
                    skip_group_check=True,
                )
                if stop:
                    emit_epilogue(blk)

        def emit_epilogue(blk):
            seg_ps = seg_tiles.pop(blk)
            segc = epi.tile([128, 2 * D], F32, tag="segc")
            nc.scalar.copy(segc[:], seg_ps[:])
            segE = segc[:, 0:D]
            segVE = segc[:, D : 2 * D]
            den = epi.tile([128, D], F32, tag="den")
            nc.gpsimd.tensor_add(den[:], segE, epsd[:])
            rec = epi.tile([128, D], F32, tag="rec")
            nc.vector.reciprocal(rec[:], den[:])
            nvb = epi.tile([128, D], F32, tag="nvb")
            nc.gpsimd.tensor_mul(nvb[:], segE, vb[:])
            num = epi.tile([128, D], F32, tag="num")
            nc.gpsimd.tensor_add(num[:], segVE, nvb[:])
            fg = epi.tile([128, D], F32, tag="fg")
            ms = epi.tile([128, 1], F32, tag="ms")
            nc.vector.scalar_tensor_tensor(
                fg[:], num[:], 1.0, rec[:],
                op0=mybir.AluOpType.mult, op1=mybir.AluOpType.mult,
                accum_out=ms[:],
            )
            mean = epi.tile([128, 1], F32, tag="mean")
            nc.vector.tensor_scalar_mul(mean[:], ms[:], 1.0 / D)
            xm = epi.tile([128, D], F32, tag="xm")
            nc.vector.tensor_scalar_sub(xm[:], fg[:], mean[:])
            sq = epi.tile([128, D], F32, tag="sq")
            vs = epi.tile([128, 1], F32, tag="vs")
            nc.vector.scalar_tensor_tensor(
                sq[:], xm[:], 1.0, xm[:],
                op0=mybir.AluOpType.mult, op1=mybir.AluOpType.mult,
                accum_out=vs[:],
            )
            tt = epi.tile([128, 1], F32, tag="tt")
            nc.vector.scalar_tensor_tensor(
                tt[:], vs[:], 1.0 / D, epsln[:],
                op0=mybir.AluOpType.mult, op1=mybir.AluOpType.add,
            )
            hh = epi.tile([128, 1], mybir.dt.uint32, tag="hh")
            nc.vector.tensor_scalar(
                hh[:], tt[:].bitcast(mybir.dt.uint32), 1, None,
                op0=mybir.AluOpType.logical_shift_right,
            )
            yy = epi.tile([128, 1], mybir.dt.uint32, tag="yy")
            nc.vector.tensor_tensor(
                yy[:], magic[:], hh[:], op=mybir.AluOpType.subtract
            )
            rs = yy[:].bitcast(F32)
            for _ in range(3):
                y2 = epi.tile([128, 1], F32, tag="y2")
                nc.vector.tensor_tensor(y2[:], rs, rs, op=mybir.AluOpType.mult)
                hty = epi.tile([128, 1], F32, tag="hty")
                nc.vector.scalar_tensor_tensor(
                    hty[:], y2[:], -0.5, tt[:],
                    op0=mybir.AluOpType.mult, op1=mybir.AluOpType.mult,
                )
                cc = epi.tile([128, 1], F32, tag="cc")
                nc.vector.tensor_scalar_add(cc[:], hty[:], 1.5)
                ny = epi.tile([128, 1], F32, tag="ny")
                nc.vector.tensor_scalar_mul(ny[:], rs, cc[:])
                rs = ny[:]
            o1 = epi.tile([128, D], F32, tag="o1")
            nc.vector.tensor_scalar_mul(o1[:], xm[:], rs)
            o2 = epi.tile([128, D], F32, tag="o2")
            nc.gpsimd.tensor_mul(o2[:], o1[:], gm[:])
            oo = epi.tile([128, D], F32, tag="oo")
            nc.gpsimd.tensor_add(oo[:], o2[:], bt[:])
            nc.sync.dma_start(y_d[blk * GBLK : (blk + 1) * GBLK, :], oo[:])

        # flat group schedule: list of (block, t_in_block)
        sched = []
        for blk in range(BPC):
            for t in range(gpbs[blk]):
                sched.append((blk, t))

        oh_tile_cur = oh_first
        for g, (blk, t) in enumerate(sched):
            if g == 0:
                ft = ft_first
            else:
                ft = ftp.tile([128, FTW], R, tag="ft")
                nc.sync.dma_start(ft[:], fT_d[:, g * FTW : (g + 1) * FTW])
            if g % OH_SUPER == 0 and g > 0:
                hi = min((g + OH_SUPER) * GBLK, TG * GBLK)
                oh_tile_cur = ohp.tile([128, OH_SUPER * GBLK], R, tag="oh")
                nc.sync.dma_start(
                    oh_tile_cur[:, 0 : hi - g * GBLK], oh_d[:, g * GBLK : hi]
                )
            oht = oh_tile_cur[:, (g % OH_SUPER) * GBLK : (g % OH_SUPER + 1) * GBLK]
            if t == 0:
                seg_tiles[blk] = seg_pool.tile(
                    [128, 2 * D], F32, name="seg", tag="seg"
                )

            st = stp.tile([128, m * 2 * D], R, tag="st")
            st3 = st[:].rearrange("p (c x) -> p c x", c=m)
            for w in range(2):
                pp = pp_pool.tile([128, 3 * 2 * D], F32)
                pp3 = pp[:].rearrange("p (c x) -> p c x", c=3)
                for s in range(3):
                    j = w * 3 + s
                    ppv = pp3[:, s, :]
                    nc.tensor.matmul(
                        ppv, ft[:, j * 128 : (j + 1) * 128], w0[:],
                        start=True, stop=False, skip_group_check=True,
                    )
                    nc.tensor.matmul(
                        ppv, ft[:, GW + j * 128 : GW + (j + 1) * 128], w1[:],
                        start=False, stop=True, skip_group_check=True,
                    )
                # one exp + one mul per 3-chunk window
                nc.scalar.activation(
                    st3[:, w * 3 : w * 3 + 3, 0:D], pp3[:, :, 0:D],
                    mybir.ActivationFunctionType.Exp,
                )
                nc.vector.tensor_mul(
                    st3[:, w * 3 : w * 3 + 3, D : 2 * D],
                    pp3[:, :, D : 2 * D],
                    st3[:, w * 3 : w * 3 + 3, 0:D],
                )
                # interleave deferred seg matmuls so the PE never waits
                # on a group's reduction tree
                if w == 1 and len(pending_seg) >= SEG_DELAY:
                    emit_seg(len(pending_seg) - SEG_DELAY + 1)

            # 6 -> 1 reduction: R1 on DVE (all-fp16 2x path), R2/R3 on Pool
            HW_ = m * D  # 1536
            r = rp.tile([128, HW_], R, tag="r")
            nc.vector.tensor_add(r[:], st[:, 0:HW_], st[:, HW_ : 2 * HW_])
            rr = rrp.tile([128, 2 * D], R, tag="rr")
            nc.gpsimd.tensor_add(rr[:], r[:, 0 : 2 * D], r[:, 2 * D : 4 * D])
            acct = accp.tile([128, 2 * D], R, name="acc", tag="acc")
            nc.gpsimd.tensor_add(acct[:], rr[:], r[:, 4 * D : 6 * D])
            pending_seg.append(
                (blk, oht, acct, t == 0, t == gpbs[blk] - 1)
            )

        emit_seg(len(pending_seg))

    _split_waits(nc)
    return nc


def _pack_host(gid, m):
    """Assign blocks (of 128 graphs) to core slots and compute per-slot
    group counts."""
    G = BPC * GBLK * N_CORES
    n_blocks = G // GBLK
    counts = np.bincount(gid, minlength=G)
    order = np.argsort(gid, kind="stable")
    g_starts = np.concatenate([[0], np.cumsum(counts)])

    lanes_per_block = np.array(
        [
            int(np.ceil(counts[b * GBLK : (b + 1) * GBLK] / m).sum())
            for b in range(n_blocks)
        ]
    )
    # slot assignment: sort blocks desc, slot k gets ranks k*8..k*8+7
    rank = np.argsort(-lanes_per_block)
    assign = np.zeros((N_CORES, BPC), np.int64)
    gpbs = []
    for k in range(BPC):
        blks = rank[k * N_CORES : (k + 1) * N_CORES]
        for c in range(N_CORES):
            assign[c, k] = blks[c]
        gpbs.append(int(np.ceil(lanes_per_block[blks].max() / 128)))
    return assign, tuple(gpbs), counts, order, g_starts


def kernel(
    f_node,
    key_W,
    key_b,
    value_W,
    value_b,
    gamma,
    beta,
    graph_id,
    num_graphs,
    trace=False,
):
    global LAST_EXEC_TIME_NS, LAST_RESULTS
    f_node = np.asarray(f_node, dtype=np.float32)
    key_W = np.asarray(key_W, dtype=np.float32)
    key_b = np.asarray(key_b, dtype=np.float32)
    value_W = np.asarray(value_W, dtype=np.float32)
    value_b = np.asarray(value_b, dtype=np.float32)
    gamma = np.asarray(gamma, dtype=np.float32)
    beta = np.asarray(beta, dtype=np.float32)
    gid = np.asarray(graph_id).astype(np.int64)
    G = int(num_graphs)
    m = PACK

    L, d = f_node.shape
    assert d == D and G == BPC * GBLK * N_CORES

    assign, gpbs, counts, order, g_starts = _pack_host(gid, m)
    TG = sum(gpbs)

    # f extended with the pad row: attn(f_pad) == PAD_ATT in every column
    f_pad = np.linalg.solve(
        key_W.astype(np.float64),
        np.full(D, PAD_ATT, np.float64) - key_b.astype(np.float64),
    ).astype(np.float32)
    f_ext = np.concatenate([f_node, f_pad[None, :]], axis=0)
    PADIDX = L

    wcat = np.ascontiguousarray(
        np.concatenate([key_W.T, value_W.T], axis=1)
    ).reshape(2, 128, 2 * D)
    vb_rep = np.ascontiguousarray(np.broadcast_to(value_b, (128, D)))
    eps_rep = np.ascontiguousarray(
        np.broadcast_to(
            (EPS_SOFTMAX / np.exp(key_b)).astype(np.float32), (128, D)
        )
    )
    gm_rep = np.ascontiguousarray(np.broadcast_to(gamma, (128, D)))
    bt_rep = np.ascontiguousarray(np.broadcast_to(beta, (128, D)))
    wcat16 = wcat.astype(np.float16)

    in_maps = []
    ymap = []  # (core, slot) -> block id
    for c in range(N_CORES):
        idx = np.full((TG, m, 128), PADIDX, np.int64)  # [group, chunk, lane]
        ohm = np.zeros((TG, 128, GBLK), np.float16)
        tg0 = 0
        for k in range(BPC):
            b = assign[c, k]
            lane = 0  # lane index within this block's group range
            for gl in range(GBLK):
                g = b * GBLK + gl
                n = counts[g]
                s = g_starts[g]
                nodes = order[s : s + n]
                for ls in range(0, n, m):
                    t = tg0 + lane // 128
                    li = lane % 128
                    seg = nodes[ls : ls + m]
                    idx[t, 0 : len(seg), li] = seg
                    ohm[t, li, gl] = 1.0
                    lane += 1
            tg0 += gpbs[k]
        # fT: [128, TG*2*GW]; col = t*(2*GW) + h*GW + j*128 + lane
        cols = idx.reshape(-1)  # [TG*m*128] node ids, (t, j, lane) order
        fshard = f_ext[cols].astype(np.float16)  # [ncols, D]
        fT = np.ascontiguousarray(
            fshard.reshape(TG, m, 128, 2, 128).transpose(4, 0, 3, 1, 2)
        ).reshape(128, TG * 2 * m * 128)
        ohT = np.ascontiguousarray(ohm.transpose(1, 0, 2)).reshape(
            128, TG * GBLK
        )
        in_maps.append(
            {
                "fT": fT,
                "oh": ohT,
                "wcat": wcat16,
                "vbrep": vb_rep,
                "epsrep": eps_rep,
                "gammarep": gm_rep,
                "betarep": bt_rep,
            }
        )
        ymap.append([assign[c, k] for k in range(BPC)])

    key = (gpbs, m)
    if key not in _nc_cache:
        _nc_cache[key] = _build_nc3(gpbs, m)
    nc = _nc_cache[key]

    if trace:
        _install_ntff_hook()
    res = run_bass_kernel_spmd(
        nc, in_maps, core_ids=list(range(N_CORES)), trace=trace
    )
    LAST_EXEC_TIME_NS = res.exec_time_ns
    LAST_RESULTS = res

    out = np.zeros((G, D), np.float32)
    for c in range(N_CORES):
        yc = res.results[c]["y"]
        for k in range(BPC):
            b = ymap[c][k]
            out[b * GBLK : (b + 1) * GBLK] = yc[k * GBLK : (k + 1) * GBLK]
    return out


def _install_ntff_hook():
    import sys, types

    try:
        if "antenv.axon_hooks" in sys.modules:
            return
        mod = types.ModuleType("antenv.axon_hooks")
        state = {"hook": None}
        mod.set_axon_ntff_profile_hook = lambda h: state.__setitem__("hook", h)
        mod.get_axon_ntff_profile_hook = lambda: state["hook"]
        sys.modules["antenv.axon_hooks"] = mod
        import antenv

        antenv.axon_hooks = mod
        from trn_agent_boot.trn_boot import _ntff_profile_via_ctypes

        mod.set_axon_ntff_profile_hook(
            _ntff_profile_via_ctypes("/opt/axon/libaxon_pjrt.so")
        )
    except Exception:
        pass


# revision 20
# speedup vs baseline: 1.0492x; 1.0492x over previous
"""AttnGraphPooling Trainium2 kernel v3 (8 NeuronCores, SPMD).

v3 vs v2 (418us -> ~332us): deeper psum pipelining + engine rebalance.
- m=6 lane packing (lane holds up to 6 nodes of one graph across the 6
  chunks of a group). Groups of 6 chunks, processed as three 2-chunk
  PSUM windows ([128, 1024] f32 = 2 banks, pool bufs=3 -> 6 banks +
  2 seg banks = 8 exactly). The 2-chunk window is the key: after a
  window's last matmul the drain chain (sem, exp ~660, sem, mul ~690)
  is ~1.6us while the PE has ~1.9us of queued fills -> the PE never
  blocks on psum reuse (3-chunk windows stalled it ~600ns/window).
- ONE fT DMA per group ([128, 1536] fp16, both D-halves concatenated)
  and one oh DMA per 4 groups -> ~1.3 DMA triggers/group (sync engine
  was 70% busy at per-chunk triggers; now 20%).
- ACT exp and DVE mul once per window (amortizes the per-instruction
  access bubbles: ~185ns ACT, ~125ns DVE). DVE is the busiest engine
  (~94%): muls (fp32-psum path, the unavoidable cost) + R1.
- Reduction 6->1 per group, emitted one group LATE so it lands on the
  in-order DVE queue behind the next group's psum-critical muls:
  R1 = st[0:3]+st[3:6] and R2 on DVE (fp16 2x path), R3 on Pool.
  One seg matmul per group accumulates oh^T @ acc into the block's
  PSUM bank, deferred SEG_DELAY=2 tree-generations so the PE rarely
  waits on the tree. Epilogue o1 runs on ACT (per-partition scale).

Padding slots use f_pad = Wk^{-1} @ (-25*ones): their attn row is ~-25
so exp -> 0 in fp16, killing their contribution (no masks). Pad lanes
additionally get an all-zero one-hot row. Epilogue is ACT-table-free
(Newton rsqrt) so the Exp table never swaps.

Measured dead ends (all regressed): 3-chunk windows (psum stall),
seg delay 3/4, ACT-staged fp16 v for 2x DVE muls (ACT queue serializes
the drain chain), R1 partially on Pool, Pool muls (Pool has no PSUM
access - compile error), fp8 anywhere (norm rel err 1.5e-2..3.8e-2 vs
the 1e-3 fp16 gets).
"""

import os as _os

import numpy as np

import concourse.bass as bass
import concourse.mybir as mybir
import concourse.tile as tile
from concourse.bass_utils import run_bass_kernel_spmd

N_CORES = 8
D = 256
GBLK = 128
BPC = 4  # blocks (of 128 graphs) per core
PACK = int(_os.environ.get("BASS_KERNEL_PACK", "6"))
SEG_DELAY = int(_os.environ.get("BASS_KERNEL_SEGDELAY", "2"))
VCOPY = int(_os.environ.get("BASS_KERNEL_VCOPY", "0"))  # windows/group staged fp16
R2_DVE = _os.environ.get("BASS_KERNEL_R2DVE", "1") == "1"
WIN = int(_os.environ.get("BASS_KERNEL_WIN", "2"))  # chunks per psum window
R1POOL = int(_os.environ.get("BASS_KERNEL_R1POOL", "0"))  # R1 cols on Pool
POOLMUL = _os.environ.get("BASS_KERNEL_POOLMUL", "0") == "1"
OH_SUPER = 4  # groups per oh DMA


def _B(name, dflt):
    return int(_os.environ.get("BASS_KERNEL_B" + name, str(dflt)))

EPS_SOFTMAX = 1e-7
EPS_LN = 1e-5
PAD_ATT = -25.0

LAST_EXEC_TIME_NS = None
LAST_RESULTS = None
_nc_cache = {}


def _split_waits(nc, maxw=1):
    cnt = 0
    for f in nc.m.functions:
        for bb in f.blocks:
            newinsts = []
            for inst in bb.instructions:
                si = getattr(inst, "sync_info", None)
                if si is not None and si.on_wait and len(si.on_wait) > maxw:
                    waits = list(si.on_wait)
                    excess = waits[:-maxw]
                    si.on_wait = waits[-maxw:]
                    for i in range(0, len(excess), maxw):
                        nop = mybir.InstNoOp(
                            name=f"Wsplit-{cnt}",
                            engine=inst.engine,
                            bass_nofuse=True,
                            sync_info=mybir.SyncInfo(
                                on_wait=excess[i : i + maxw], on_update=[]
                            ),
                        )
                        cnt += 1
                        newinsts.append(nop)
                newinsts.append(inst)
            bb.instructions = newinsts
    return cnt


def _build_nc3(gpbs, m):
    """gpbs: tuple of groups-per-block for the BPC block slots (same on
    every core); m: lane depth (chunks per group). m must be 6."""
    from contextlib import ExitStack

    R = mybir.dt.float16
    F32 = mybir.dt.float32
    TG = sum(gpbs)
    assert m == 6
    GW = m * 128  # node cols per group (one half)
    FTW = 2 * GW  # fT cols per group (both halves)

    nc = bass.Bass()
    fT_d = nc.dram_tensor("fT", [128, TG * FTW], R, kind="ExternalInput")
    oh_d = nc.dram_tensor("oh", [128, TG * GBLK], R, kind="ExternalInput")
    wcat_d = nc.dram_tensor("wcat", [2, 128, 2 * D], R, kind="ExternalInput")
    vb_d = nc.dram_tensor("vbrep", [128, D], F32, kind="ExternalInput")
    epsd_d = nc.dram_tensor("epsrep", [128, D], F32, kind="ExternalInput")
    gm_d = nc.dram_tensor("gammarep", [128, D], F32, kind="ExternalInput")
    bt_d = nc.dram_tensor("betarep", [128, D], F32, kind="ExternalInput")
    y_d = nc.dram_tensor("y", [BPC * GBLK, D], F32, kind="ExternalOutput")

    with tile.TileContext(nc) as tc, ExitStack() as ctx:
        const = ctx.enter_context(tc.tile_pool(name="const", bufs=1))
        ftp = ctx.enter_context(tc.tile_pool(name="ft", bufs=3))
        ohp = ctx.enter_context(tc.tile_pool(name="oh", bufs=_B("OH", 3)))
        stp = ctx.enter_context(tc.tile_pool(name="st", bufs=_B("ST", 4)))
        rp = ctx.enter_context(tc.tile_pool(name="r", bufs=_B("R", 4)))
        rrp = ctx.enter_context(tc.tile_pool(name="rr", bufs=_B("RR", 3)))
        vcp = ctx.enter_context(tc.tile_pool(name="vc", bufs=_B("VC", 2)))
        accp = ctx.enter_context(tc.tile_pool(name="acc", bufs=_B("ACC", 4)))
        epi = ctx.enter_context(tc.tile_pool(name="epi", bufs=2))
        pp_pool = ctx.enter_context(
            tc.tile_pool(name="pp", bufs=6 // WIN, space="PSUM")
        )
        seg_pool = ctx.enter_context(tc.tile_pool(name="seg", bufs=2, space="PSUM"))

        # first fT tile first so the PE starts ASAP
        ft_first = ftp.tile([128, FTW], R, tag="ft")
        nc.sync.dma_start(ft_first[:], fT_d[:, 0:FTW])
        w0 = const.tile([128, 2 * D], R, tag="w0")
        nc.sync.dma_start(w0[:], wcat_d[0])
        w1 = const.tile([128, 2 * D], R, tag="w1")
        nc.sync.dma_start(w1[:], wcat_d[1])
        oh_first = ohp.tile([128, OH_SUPER * GBLK], R, tag="oh")
        nc.sync.dma_start(oh_first[:], oh_d[:, 0 : OH_SUPER * GBLK])
        vb = const.tile([128, D], F32, tag="vb")
        nc.sync.dma_start(vb[:], vb_d[:])
        epsd = const.tile([128, D], F32, tag="epsd")
        nc.sync.dma_start(epsd[:], epsd_d[:])
        gm = const.tile([128, D], F32, tag="gm")
        nc.sync.dma_start(gm[:], gm_d[:])
        bt = const.tile([128, D], F32, tag="bt")
        nc.sync.dma_start(bt[:], bt_d[:])
        epsln = const.tile([128, 1], F32, tag="epsln")
        nc.gpsimd.memset(epsln[:], float(EPS_LN))
        magic = const.tile([128, 1], mybir.dt.uint32, tag="magic")
        nc.gpsimd.memset(magic[:], 0x5F3759DF)

        warm = const.tile([128, 1], F32, tag="warm")
        warm2 = const.tile([128, 1], F32, tag="warm2")
        nc.gpsimd.memset(warm[:], 1.0)
        nc.scalar.activation(warm2[:], warm[:], mybir.ActivationFunctionType.Exp)

        seg_tiles = {}
        pending_seg = []
        pending_tree = []

        def emit_seg(n):
            # emit up to n queued seg matmuls (oldest first)
            for _ in range(min(n, len(pending_seg))):
                blk, oht, acct, start, stop = pending_seg.pop(0)
                nc.tensor.matmul(
                    seg_tiles[blk][:],
                    oht,
                    acct[:],
                    start=start,
                    stop=stop,
                    skip_group_check=True,
                )
                if stop:
                    emit_epilogue(blk)

        def emit_epilogue(blk):
            seg_ps = seg_tiles.pop(blk)
            segc = epi.tile([128, 2 * D], F32, tag="segc")
            nc.scalar.copy(segc[:], seg_ps[:])
            segE = segc[:, 0:D]
            segVE = segc[:, D : 2 * D]
            den = epi.tile([128, D], F32, tag="den")
            nc.gpsimd.tensor_add(den[:], segE, epsd[:])
            rec = epi.tile([128, D], F32, tag="rec")
            nc.vector.reciprocal(rec[:], den[:])
            nvb = epi.tile([128, D], F32, tag="nvb")
            nc.gpsimd.tensor_mul(nvb[:], segE, vb[:])
            num = epi.tile([128, D], F32, tag="num")
            nc.gpsimd.tensor_add(num[:], segVE, nvb[:])
            fg = epi.tile([128, D], F32, tag="fg")
            ms = epi.tile([128, 1], F32, tag="ms")
            nc.vector.scalar_tensor_tensor(
                fg[:], num[:], 1.0, rec[:],
                op0=mybir.AluOpType.mult, op1=mybir.AluOpType.mult,
                accum_out=ms[:],
            )
            mean = epi.tile([128, 1], F32, tag="mean")
            nc.vector.tensor_scalar_mul(mean[:], ms[:], 1.0 / D)
            xm = epi.tile([128, D], F32, tag="xm")
            nc.vector.tensor_scalar_sub(xm[:], fg[:], mean[:])
            sq = epi.tile([128, D], F32, tag="sq")
            vs = epi.tile([128, 1], F32, tag="vs")
            nc.vector.scalar_tensor_tensor(
                sq[:], xm[:], 1.0, xm[:],
                op0=mybir.AluOpType.mult, op1=mybir.AluOpType.mult,
                accum_out=vs[:],
            )
            tt = epi.tile([128, 1], F32, tag="tt")
            nc.vector.scalar_tensor_tensor(
                tt[:], vs[:], 1.0 / D, epsln[:],
                op0=mybir.AluOpType.mult, op1=mybir.AluOpType.add,
            )
            hh = epi.tile([128, 1], mybir.dt.uint32, tag="hh")
            nc.vector.tensor_scalar(
                hh[:], tt[:].bitcast(mybir.dt.uint32), 1, None,
                op0=mybir.AluOpType.logical_shift_right,
            )
            yy = epi.tile([128, 1], mybir.dt.uint32, tag="yy")
            nc.vector.tensor_tensor(
                yy[:], magic[:], hh[:], op=mybir.AluOpType.subtract
            )
            rs = yy[:].bitcast(F32)
            for _ in range(3):
                y2 = epi.tile([128, 1], F32, tag="y2")
                nc.vector.tensor_tensor(y2[:], rs, rs, op=mybir.AluOpType.mult)
                hty = epi.tile([128, 1], F32, tag="hty")
                nc.vector.scalar_tensor_tensor(
                    hty[:], y2[:], -0.5, tt[:],
                    op0=mybir.AluOpType.mult, op1=mybir.AluOpType.mult,
                )
                cc = epi.tile([128, 1], F32, tag="cc")
                nc.vector.tensor_scalar_add(cc[:], hty[:], 1.5)
                ny = epi.tile([128, 1], F32, tag="ny")
                nc.vector.tensor_scalar_mul(ny[:], rs, cc[:])
                rs = ny[:]
            o1 = epi.tile([128, D], F32, tag="o1")
            nc.scalar.mul(o1[:], xm[:], rs)
            o2 = epi.tile([128, D], F32, tag="o2")
            nc.gpsimd.tensor_mul(o2[:], o1[:], gm[:])
            oo = epi.tile([128, D], F32, tag="oo")
            nc.gpsimd.tensor_add(oo[:], o2[:], bt[:])
            nc.sync.dma_start(y_d[blk * GBLK : (blk + 1) * GBLK, :], oo[:])

        # flat group schedule: list of (block, t_in_block)
        sched = []
        for blk in range(BPC):
            for t in range(gpbs[blk]):
                sched.append((blk, t))

        oh_tile_cur = oh_first
        for g, (blk, t) in enumerate(sched):
            if g == 0:
                ft = ft_first
            else:
                ft = ftp.tile([128, FTW], R, tag="ft")
                nc.sync.dma_start(ft[:], fT_d[:, g * FTW : (g + 1) * FTW])
            if g % OH_SUPER == 0 and g > 0:
                hi = min((g + OH_SUPER) * GBLK, TG * GBLK)
                oh_tile_cur = ohp.tile([128, OH_SUPER * GBLK], R, tag="oh")
                nc.sync.dma_start(
                    oh_tile_cur[:, 0 : hi - g * GBLK], oh_d[:, g * GBLK : hi]
                )
            oht = oh_tile_cur[:, (g % OH_SUPER) * GBLK : (g % OH_SUPER + 1) * GBLK]
            if t == 0:
                seg_tiles[blk] = seg_pool.tile(
                    [128, 2 * D], F32, name="seg", tag="seg"
                )

            def emit_tree(n):
                # emit up to n queued reduction trees (oldest first).
                # R1+R2 back-to-back on DVE (inputs a full group old, no
                # queue-head blocking), only R3 hops to Pool - keeps the
                # acc latency short so the seg matmul never waits.
                for _ in range(min(n, len(pending_tree))):
                    pst, pblk, poht, pstart, pstop = pending_tree.pop(0)
                    HW_ = m * D  # 1536
                    r = rp.tile([128, HW_], R, tag="r")
                    if R1POOL > 0:
                        x = HW_ - R1POOL
                        nc.vector.tensor_add(
                            r[:, 0:x], pst[:, 0:x], pst[:, HW_ : HW_ + x]
                        )
                        nc.gpsimd.tensor_add(
                            r[:, x:HW_], pst[:, x:HW_], pst[:, HW_ + x : 2 * HW_]
                        )
                    else:
                        nc.vector.tensor_add(
                            r[:], pst[:, 0:HW_], pst[:, HW_ : 2 * HW_]
                        )
                    rr = rrp.tile([128, 2 * D], R, tag="rr")
                    if R2_DVE:
                        nc.vector.tensor_add(
                            rr[:], r[:, 0 : 2 * D], r[:, 2 * D : 4 * D]
                        )
                    else:
                        nc.gpsimd.tensor_add(
                            rr[:], r[:, 0 : 2 * D], r[:, 2 * D : 4 * D]
                        )
                    acct = accp.tile([128, 2 * D], R, name="acc", tag="acc")
                    nc.gpsimd.tensor_add(acct[:], rr[:], r[:, 4 * D : 6 * D])
                    pending_seg.append((pblk, poht, acct, pstart, pstop))

            st = stp.tile([128, m * 2 * D], R, tag="st")
            st3 = st[:].rearrange("p (c x) -> p c x", c=m)
            n_win = m // WIN
            for w in range(n_win):
                pp = pp_pool.tile([128, WIN * 2 * D], F32)
                pp3 = pp[:].rearrange("p (c x) -> p c x", c=WIN)
                for s in range(WIN):
                    j = w * WIN + s
                    ppv = pp3[:, s, :]
                    nc.tensor.matmul(
                        ppv, ft[:, j * 128 : (j + 1) * 128], w0[:],
                        start=True, stop=False, skip_group_check=True,
                    )
                    nc.tensor.matmul(
                        ppv, ft[:, GW + j * 128 : GW + (j + 1) * 128], w1[:],
                        start=False, stop=True, skip_group_check=True,
                    )
                # one exp + one mul per window
                nc.scalar.activation(
                    st3[:, w * WIN : (w + 1) * WIN, 0:D], pp3[:, :, 0:D],
                    mybir.ActivationFunctionType.Exp,
                )
                if POOLMUL and w == n_win - 1:
                    # last window's multiply on Pool: DVE relief; Pool
                    # frees the psum window inside the reuse slack
                    nc.gpsimd.tensor_mul(
                        st3[:, w * WIN : (w + 1) * WIN, D : 2 * D],
                        pp3[:, :, D : 2 * D],
                        st3[:, w * WIN : (w + 1) * WIN, 0:D],
                    )
                elif w < VCOPY:
                    # ACT stages v to fp16 SBUF so the DVE multiply runs
                    # all-2-byte (2x path); relieves the DVE backlog
                    vc = vcp.tile([128, WIN * D], R, tag="vc")
                    vc3 = vc[:].rearrange("p (c x) -> p c x", c=WIN)
                    nc.scalar.copy(vc3[:, :, :], pp3[:, :, D : 2 * D])
                    nc.vector.tensor_mul(
                        st3[:, w * WIN : (w + 1) * WIN, D : 2 * D],
                        vc3[:, :, :],
                        st3[:, w * WIN : (w + 1) * WIN, 0:D],
                    )
                else:
                    nc.vector.tensor_mul(
                        st3[:, w * WIN : (w + 1) * WIN, D : 2 * D],
                        pp3[:, :, D : 2 * D],
                        st3[:, w * WIN : (w + 1) * WIN, 0:D],
                    )
                if w == n_win - 1 and len(pending_seg) >= SEG_DELAY:
                    emit_seg(len(pending_seg) - SEG_DELAY + 1)

            # previous group's tree at end-of-group: R1 never delays the
            # psum-critical muls on the in-order DVE queue
            emit_tree(1)
            pending_tree.append(
                (st, blk, oht, t == 0, t == gpbs[blk] - 1)
            )

        while pending_tree or pending_seg:
            emit_tree(1)
            emit_seg(len(pending_seg))

    _split_waits(nc)
    return nc


def _pack_host(gid, m):
    """Assign blocks (of 128 graphs) to core slots and compute per-slot
    group counts."""
    G = BPC * GBLK * N_CORES
    n_blocks = G // GBLK
    counts = np.bincount(gid, minlength=G)
    order = np.argsort(gid, kind="stable")
    g_starts = np.concatenate([[0], np.cumsum(counts)])

    lanes_per_block = np.array(
        [
            int(np.ceil(counts[b * GBLK : (b + 1) * GBLK] / m).sum())
            for b in range(n_blocks)
        ]
    )
    # slot assignment: sort blocks desc, slot k gets ranks k*8..k*8+7
    rank = np.argsort(-lanes_per_block)
    assign = np.zeros((N_CORES, BPC), np.int64)
    gpbs = []
    for k in range(BPC):
        blks = rank[k * N_CORES : (k + 1) * N_CORES]
        for c in range(N_CORES):
            assign[c, k] = blks[c]
        gpbs.append(int(np.ceil(lanes_per_block[blks].max() / 128)))
    return assign, tuple(gpbs), counts, order, g_starts


def kernel(
    f_node,
    key_W,
    key_b,
    value_W,
    value_b,
    gamma,
    beta,
    graph_id,
    num_graphs,
    trace=False,
):
    global LAST_EXEC_TIME_NS, LAST_RESULTS
    f_node = np.asarray(f_node, dtype=np.float32)
    key_W = np.asarray(key_W, dtype=np.float32)
    key_b = np.asarray(key_b, dtype=np.float32)
    value_W = np.asarray(value_W, dtype=np.float32)
    value_b = np.asarray(value_b, dtype=np.float32)
    gamma = np.asarray(gamma, dtype=np.float32)
    beta = np.asarray(beta, dtype=np.float32)
    gid = np.asarray(graph_id).astype(np.int64)
    G = int(num_graphs)
    m = PACK

    L, d = f_node.shape
    assert d == D and G == BPC * GBLK * N_CORES

    assign, gpbs, counts, order, g_starts = _pack_host(gid, m)
    TG = sum(gpbs)

    # f extended with the pad row: attn(f_pad) == PAD_ATT in every column
    f_pad = np.linalg.solve(
        key_W.astype(np.float64),
        np.full(D, PAD_ATT, np.float64) - key_b.astype(np.float64),
    ).astype(np.float32)
    f_ext = np.concatenate([f_node, f_pad[None, :]], axis=0)
    PADIDX = L

    wcat = np.ascontiguousarray(
        np.concatenate([key_W.T, value_W.T], axis=1)
    ).reshape(2, 128, 2 * D)
    vb_rep = np.ascontiguousarray(np.broadcast_to(value_b, (128, D)))
    eps_rep = np.ascontiguousarray(
        np.broadcast_to(
            (EPS_SOFTMAX / np.exp(key_b)).astype(np.float32), (128, D)
        )
    )
    gm_rep = np.ascontiguousarray(np.broadcast_to(gamma, (128, D)))
    bt_rep = np.ascontiguousarray(np.broadcast_to(beta, (128, D)))
    wcat16 = wcat.astype(np.float16)

    in_maps = []
    ymap = []  # (core, slot) -> block id
    for c in range(N_CORES):
        idx = np.full((TG, m, 128), PADIDX, np.int64)  # [group, chunk, lane]
        ohm = np.zeros((TG, 128, GBLK), np.float16)
        tg0 = 0
        for k in range(BPC):
            b = assign[c, k]
            lane = 0  # lane index within this block's group range
            for gl in range(GBLK):
                g = b * GBLK + gl
                n = counts[g]
                s = g_starts[g]
                nodes = order[s : s + n]
                for ls in range(0, n, m):
                    t = tg0 + lane // 128
                    li = lane % 128
                    seg = nodes[ls : ls + m]
                    idx[t, 0 : len(seg), li] = seg
                    ohm[t, li, gl] = 1.0
                    lane += 1
            tg0 += gpbs[k]
        # fT: [128, TG*2*GW]; col = t*(2*GW) + h*GW + j*128 + lane
        cols = idx.reshape(-1)  # [TG*m*128] node ids, (t, j, lane) order
        fshard = f_ext[cols].astype(np.float16)  # [ncols, D]
        fT = np.ascontiguousarray(
            fshard.reshape(TG, m, 128, 2, 128).transpose(4, 0, 3, 1, 2)
        ).reshape(128, TG * 2 * m * 128)
        ohT = np.ascontiguousarray(ohm.transpose(1, 0, 2)).reshape(
            128, TG * GBLK
        )
        in_maps.append(
            {
                "fT": fT,
                "oh": ohT,
                "wcat": wcat16,
                "vbrep": vb_rep,
                "epsrep": eps_rep,
                "gammarep": gm_rep,
                "betarep": bt_rep,
            }
        )
        ymap.append([assign[c, k] for k in range(BPC)])

    key = (gpbs, m)
    if key not in _nc_cache:
        _nc_cache[key] = _build_nc3(gpbs, m)
    nc = _nc_cache[key]

    if trace:
        _install_ntff_hook()
    res = run_bass_kernel_spmd(
        nc, in_maps, core_ids=list(range(N_CORES)), trace=trace
    )
    LAST_EXEC_TIME_NS = res.exec_time_ns
    LAST_RESULTS = res

    out = np.zeros((G, D), np.float32)
    for c in range(N_CORES):
        yc = res.results[c]["y"]
        for k in range(BPC):
            b = ymap[c][k]
            out[b * GBLK : (b + 1) * GBLK] = yc[k * GBLK : (k + 1) * GBLK]
    return out


def _install_ntff_hook():
    import sys, types

    try:
        if "antenv.axon_hooks" in sys.modules:
            return
        mod = types.ModuleType("antenv.axon_hooks")
        state = {"hook": None}
        mod.set_axon_ntff_profile_hook = lambda h: state.__setitem__("hook", h)
        mod.get_axon_ntff_profile_hook = lambda: state["hook"]
        sys.modules["antenv.axon_hooks"] = mod
        import antenv

        antenv.axon_hooks = mod
        from trn_agent_boot.trn_boot import _ntff_profile_via_ctypes

        mod.set_axon_ntff_profile_hook(
            _ntff_profile_via_ctypes("/opt/axon/libaxon_pjrt.so")
        )
    except Exception:
        pass


# revision 21
# speedup vs baseline: 1.2482x; 1.1896x over previous
"""AttnGraphPooling Trainium2 kernel v3 (8 NeuronCores, SPMD).

v3 vs v2 (418us -> ~332us): deeper psum pipelining + engine rebalance.
- m=6 lane packing (lane holds up to 6 nodes of one graph across the 6
  chunks of a group). Groups of 6 chunks, processed as three 2-chunk
  PSUM windows ([128, 1024] f32 = 2 banks, pool bufs=3 -> 6 banks +
  2 seg banks = 8 exactly). The 2-chunk window is the key: after a
  window's last matmul the drain chain (sem, exp ~660, sem, mul ~690)
  is ~1.6us while the PE has ~1.9us of queued fills -> the PE never
  blocks on psum reuse (3-chunk windows stalled it ~600ns/window).
- ONE fT DMA per group ([128, 1536] fp16, both D-halves concatenated)
  and one oh DMA per 4 groups -> ~1.3 DMA triggers/group (sync engine
  was 70% busy at per-chunk triggers; now 20%).
- ACT exp and DVE mul once per window (amortizes the per-instruction
  access bubbles: ~185ns ACT, ~125ns DVE). DVE is the busiest engine
  (~94%): muls (fp32-psum path, the unavoidable cost) + R1.
- Reduction 6->1 per group, emitted one group LATE so it lands on the
  in-order DVE queue behind the next group's psum-critical muls:
  R1 = st[0:3]+st[3:6] and R2 on DVE (fp16 2x path), R3 on Pool.
  One seg matmul per group accumulates oh^T @ acc into the block's
  PSUM bank, deferred SEG_DELAY=2 tree-generations so the PE rarely
  waits on the tree. Epilogue o1 runs on ACT (per-partition scale).

Padding slots use f_pad = Wk^{-1} @ (-25*ones): their attn row is ~-25
so exp -> 0 in fp16, killing their contribution (no masks). Pad lanes
additionally get an all-zero one-hot row. Epilogue is ACT-table-free
(Newton rsqrt) so the Exp table never swaps.

Measured dead ends (all regressed): 3-chunk windows (psum stall),
seg delay 3/4, ACT-staged fp16 v for 2x DVE muls (ACT queue serializes
the drain chain), R1 partially on Pool, Pool muls (Pool has no PSUM
access - compile error), fp8 anywhere (norm rel err 1.5e-2..3.8e-2 vs
the 1e-3 fp16 gets).
"""

import os as _os

import numpy as np

import concourse.bass as bass
import concourse.mybir as mybir
import concourse.tile as tile
from concourse.bass_utils import run_bass_kernel_spmd

N_CORES = 8
D = 256
GBLK = 128
BPC = 4  # blocks (of 128 graphs) per core
PACK = int(_os.environ.get("BASS_KERNEL_PACK", "6"))
SEG_DELAY = int(_os.environ.get("BASS_KERNEL_SEGDELAY", "2"))
VCOPY = int(_os.environ.get("BASS_KERNEL_VCOPY", "0"))  # windows/group staged fp16
R2_DVE = _os.environ.get("BASS_KERNEL_R2DVE", "1") == "1"
WIN = int(_os.environ.get("BASS_KERNEL_WIN", "2"))  # chunks per psum window
R1POOL = int(_os.environ.get("BASS_KERNEL_R1POOL", "0"))  # R1 cols on Pool
POOLMUL = _os.environ.get("BASS_KERNEL_POOLMUL", "0") == "1"
OH_SUPER = 4  # groups per oh DMA


def _B(name, dflt):
    return int(_os.environ.get("BASS_KERNEL_B" + name, str(dflt)))

EPS_SOFTMAX = 1e-7
EPS_LN = 1e-5
PAD_ATT = -25.0

LAST_EXEC_TIME_NS = None
LAST_RESULTS = None
_nc_cache = {}


def _split_waits(nc, maxw=1):
    cnt = 0
    for f in nc.m.functions:
        for bb in f.blocks:
            newinsts = []
            for inst in bb.instructions:
                si = getattr(inst, "sync_info", None)
                if si is not None and si.on_wait and len(si.on_wait) > maxw:
                    waits = list(si.on_wait)
                    excess = waits[:-maxw]
                    si.on_wait = waits[-maxw:]
                    for i in range(0, len(excess), maxw):
                        nop = mybir.InstNoOp(
                            name=f"Wsplit-{cnt}",
                            engine=inst.engine,
                            bass_nofuse=True,
                            sync_info=mybir.SyncInfo(
                                on_wait=excess[i : i + maxw], on_update=[]
                            ),
                        )
                        cnt += 1
                        newinsts.append(nop)
                newinsts.append(inst)
            bb.instructions = newinsts
    return cnt


def _build_nc3(gpbs, m):
    """gpbs: tuple of groups-per-block for the BPC block slots (same on
    every core); m: lane depth (chunks per group). m must be 6."""
    from contextlib import ExitStack

    R = mybir.dt.float16
    F32 = mybir.dt.float32
    TG = sum(gpbs)
    assert m == 6
    GW = m * 128  # node cols per group (one half)
    FTW = 2 * GW  # fT cols per group (both halves)

    nc = bass.Bass()
    fT_d = nc.dram_tensor("fT", [128, TG * FTW], R, kind="ExternalInput")
    oh_d = nc.dram_tensor("oh", [128, TG * GBLK], R, kind="ExternalInput")
    wcat_d = nc.dram_tensor("wcat", [2, 128, 2 * D], R, kind="ExternalInput")
    vb_d = nc.dram_tensor("vbrep", [128, D], F32, kind="ExternalInput")
    epsd_d = nc.dram_tensor("epsrep", [128, D], F32, kind="ExternalInput")
    gm_d = nc.dram_tensor("gammarep", [128, D], F32, kind="ExternalInput")
    bt_d = nc.dram_tensor("betarep", [128, D], F32, kind="ExternalInput")
    y_d = nc.dram_tensor("y", [BPC * GBLK, D], F32, kind="ExternalOutput")

    with tile.TileContext(nc) as tc, ExitStack() as ctx:
        const = ctx.enter_context(tc.tile_pool(name="const", bufs=1))
        ftp = ctx.enter_context(tc.tile_pool(name="ft", bufs=3))
        ohp = ctx.enter_context(tc.tile_pool(name="oh", bufs=_B("OH", 3)))
        stp = ctx.enter_context(tc.tile_pool(name="st", bufs=_B("ST", 3)))
        rp = ctx.enter_context(tc.tile_pool(name="r", bufs=_B("R", 3)))
        rrp = ctx.enter_context(tc.tile_pool(name="rr", bufs=_B("RR", 3)))
        vcp = ctx.enter_context(tc.tile_pool(name="vc", bufs=_B("VC", 2)))
        accp = ctx.enter_context(tc.tile_pool(name="acc", bufs=_B("ACC", 6)))
        epi = ctx.enter_context(tc.tile_pool(name="epi", bufs=2))
        pp_pool = ctx.enter_context(
            tc.tile_pool(name="pp", bufs=6 // WIN, space="PSUM")
        )
        seg_pool = ctx.enter_context(tc.tile_pool(name="seg", bufs=2, space="PSUM"))

        # first fT tile first so the PE starts ASAP
        ft_first = ftp.tile([128, FTW], R, tag="ft")
        nc.sync.dma_start(ft_first[:], fT_d[:, 0:FTW])
        w0 = const.tile([128, 2 * D], R, tag="w0")
        nc.sync.dma_start(w0[:], wcat_d[0])
        w1 = const.tile([128, 2 * D], R, tag="w1")
        nc.sync.dma_start(w1[:], wcat_d[1])
        oh_first = ohp.tile([128, OH_SUPER * GBLK], R, tag="oh")
        nc.sync.dma_start(oh_first[:], oh_d[:, 0 : OH_SUPER * GBLK])
        vb = const.tile([128, D], F32, tag="vb")
        nc.sync.dma_start(vb[:], vb_d[:])
        epsd = const.tile([128, D], F32, tag="epsd")
        nc.sync.dma_start(epsd[:], epsd_d[:])
        gm = const.tile([128, D], F32, tag="gm")
        nc.sync.dma_start(gm[:], gm_d[:])
        bt = const.tile([128, D], F32, tag="bt")
        nc.sync.dma_start(bt[:], bt_d[:])
        epsln = const.tile([128, 1], F32, tag="epsln")
        nc.gpsimd.memset(epsln[:], float(EPS_LN))
        magic = const.tile([128, 1], mybir.dt.uint32, tag="magic")
        nc.gpsimd.memset(magic[:], 0x5F3759DF)

        warm = const.tile([128, 1], F32, tag="warm")
        warm2 = const.tile([128, 1], F32, tag="warm2")
        nc.gpsimd.memset(warm[:], 1.0)
        nc.scalar.activation(warm2[:], warm[:], mybir.ActivationFunctionType.Exp)

        seg_tiles = {}
        pending_seg = []
        pending_tree = []

        def emit_seg(n):
            # emit up to n queued seg matmuls (oldest first)
            for _ in range(min(n, len(pending_seg))):
                blk, oht, acct, start, stop = pending_seg.pop(0)
                nc.tensor.matmul(
                    seg_tiles[blk][:],
                    oht,
                    acct[:],
                    start=start,
                    stop=stop,
                    skip_group_check=True,
                )
                if stop:
                    emit_epilogue(blk)

        def emit_epilogue(blk):
            seg_ps = seg_tiles.pop(blk)
            segc = epi.tile([128, 2 * D], F32, tag="segc")
            nc.scalar.copy(segc[:], seg_ps[:])
            segE = segc[:, 0:D]
            segVE = segc[:, D : 2 * D]
            den = epi.tile([128, D], F32, tag="den")
            nc.gpsimd.tensor_add(den[:], segE, epsd[:])
            rec = epi.tile([128, D], F32, tag="rec")
            nc.vector.reciprocal(rec[:], den[:])
            nvb = epi.tile([128, D], F32, tag="nvb")
            nc.gpsimd.tensor_mul(nvb[:], segE, vb[:])
            num = epi.tile([128, D], F32, tag="num")
            nc.gpsimd.tensor_add(num[:], segVE, nvb[:])
            fg = epi.tile([128, D], F32, tag="fg")
            ms = epi.tile([128, 1], F32, tag="ms")
            nc.vector.scalar_tensor_tensor(
                fg[:], num[:], 1.0, rec[:],
                op0=mybir.AluOpType.mult, op1=mybir.AluOpType.mult,
                accum_out=ms[:],
            )
            mean = epi.tile([128, 1], F32, tag="mean")
            nc.vector.tensor_scalar_mul(mean[:], ms[:], 1.0 / D)
            xm = epi.tile([128, D], F32, tag="xm")
            nc.vector.tensor_scalar_sub(xm[:], fg[:], mean[:])
            sq = epi.tile([128, D], F32, tag="sq")
            vs = epi.tile([128, 1], F32, tag="vs")
            nc.vector.scalar_tensor_tensor(
                sq[:], xm[:], 1.0, xm[:],
                op0=mybir.AluOpType.mult, op1=mybir.AluOpType.mult,
                accum_out=vs[:],
            )
            tt = epi.tile([128, 1], F32, tag="tt")
            nc.vector.scalar_tensor_tensor(
                tt[:], vs[:], 1.0 / D, epsln[:],
                op0=mybir.AluOpType.mult, op1=mybir.AluOpType.add,
            )
            hh = epi.tile([128, 1], mybir.dt.uint32, tag="hh")
            nc.vector.tensor_scalar(
                hh[:], tt[:].bitcast(mybir.dt.uint32), 1, None,
                op0=mybir.AluOpType.logical_shift_right,
            )
            yy = epi.tile([128, 1], mybir.dt.uint32, tag="yy")
            nc.vector.tensor_tensor(
                yy[:], magic[:], hh[:], op=mybir.AluOpType.subtract
            )
            rs = yy[:].bitcast(F32)
            for _ in range(3):
                y2 = epi.tile([128, 1], F32, tag="y2")
                nc.vector.tensor_tensor(y2[:], rs, rs, op=mybir.AluOpType.mult)
                hty = epi.tile([128, 1], F32, tag="hty")
                nc.vector.scalar_tensor_tensor(
                    hty[:], y2[:], -0.5, tt[:],
                    op0=mybir.AluOpType.mult, op1=mybir.AluOpType.mult,
                )
                cc = epi.tile([128, 1], F32, tag="cc")
                nc.vector.tensor_scalar_add(cc[:], hty[:], 1.5)
                ny = epi.tile([128, 1], F32, tag="ny")
                nc.vector.tensor_scalar_mul(ny[:], rs, cc[:])
                rs = ny[:]
            o1 = epi.tile([128, D], F32, tag="o1")
            nc.scalar.mul(o1[:], xm[:], rs)
            o2 = epi.tile([128, D], F32, tag="o2")
            nc.gpsimd.tensor_mul(o2[:], o1[:], gm[:])
            oo = epi.tile([128, D], F32, tag="oo")
            nc.gpsimd.tensor_add(oo[:], o2[:], bt[:])
            nc.sync.dma_start(y_d[blk * GBLK : (blk + 1) * GBLK, :], oo[:])

        # flat group schedule: list of (block, t_in_block)
        sched = []
        for blk in range(BPC):
            for t in range(gpbs[blk]):
                sched.append((blk, t))

        oh_tile_cur = oh_first
        for g, (blk, t) in enumerate(sched):
            if g == 0:
                ft = ft_first
            else:
                ft = ftp.tile([128, FTW], R, tag="ft")
                nc.sync.dma_start(ft[:], fT_d[:, g * FTW : (g + 1) * FTW])
            if g % OH_SUPER == 0 and g > 0:
                hi = min((g + OH_SUPER) * GBLK, TG * GBLK)
                oh_tile_cur = ohp.tile([128, OH_SUPER * GBLK], R, tag="oh")
                nc.sync.dma_start(
                    oh_tile_cur[:, 0 : hi - g * GBLK], oh_d[:, g * GBLK : hi]
                )
            oht = oh_tile_cur[:, (g % OH_SUPER) * GBLK : (g % OH_SUPER + 1) * GBLK]
            if t == 0:
                seg_tiles[blk] = seg_pool.tile(
                    [128, 2 * D], F32, name="seg", tag="seg"
                )

            def emit_tree(n):
                # emit up to n queued reduction trees (oldest first).
                # R1+R2 back-to-back on DVE (inputs a full group old, no
                # queue-head blocking), only R3 hops to Pool - keeps the
                # acc latency short so the seg matmul never waits.
                for _ in range(min(n, len(pending_tree))):
                    pst, pblk, poht, pstart, pstop = pending_tree.pop(0)
                    HW_ = m * D  # 1536
                    r = rp.tile([128, HW_], R, tag="r")
                    if R1POOL > 0:
                        x = HW_ - R1POOL
                        nc.vector.tensor_add(
                            r[:, 0:x], pst[:, 0:x], pst[:, HW_ : HW_ + x]
                        )
                        nc.gpsimd.tensor_add(
                            r[:, x:HW_], pst[:, x:HW_], pst[:, HW_ + x : 2 * HW_]
                        )
                    else:
                        nc.vector.tensor_add(
                            r[:], pst[:, 0:HW_], pst[:, HW_ : 2 * HW_]
                        )
                    rr = rrp.tile([128, 2 * D], R, tag="rr")
                    if R2_DVE:
                        nc.vector.tensor_add(
                            rr[:], r[:, 0 : 2 * D], r[:, 2 * D : 4 * D]
                        )
                    else:
                        nc.gpsimd.tensor_add(
                            rr[:], r[:, 0 : 2 * D], r[:, 2 * D : 4 * D]
                        )
                    acct = accp.tile([128, 2 * D], R, name="acc", tag="acc")
                    nc.gpsimd.tensor_add(acct[:], rr[:], r[:, 4 * D : 6 * D])
                    pending_seg.append((pblk, poht, acct, pstart, pstop))

            st = stp.tile([128, m * 2 * D], R, tag="st")
            st3 = st[:].rearrange("p (c x) -> p c x", c=m)
            n_win = m // WIN
            for w in range(n_win):
                pp = pp_pool.tile([128, WIN * 2 * D], F32)
                pp3 = pp[:].rearrange("p (c x) -> p c x", c=WIN)
                for s in range(WIN):
                    j = w * WIN + s
                    ppv = pp3[:, s, :]
                    nc.tensor.matmul(
                        ppv, ft[:, j * 128 : (j + 1) * 128], w0[:],
                        start=True, stop=False, skip_group_check=True,
                    )
                    nc.tensor.matmul(
                        ppv, ft[:, GW + j * 128 : GW + (j + 1) * 128], w1[:],
                        start=False, stop=True, skip_group_check=True,
                    )
                # one exp + one mul per window
                nc.scalar.activation(
                    st3[:, w * WIN : (w + 1) * WIN, 0:D], pp3[:, :, 0:D],
                    mybir.ActivationFunctionType.Exp,
                )
                if POOLMUL and w == n_win - 1:
                    # last window's multiply on Pool: DVE relief; Pool
                    # frees the psum window inside the reuse slack
                    nc.gpsimd.tensor_mul(
                        st3[:, w * WIN : (w + 1) * WIN, D : 2 * D],
                        pp3[:, :, D : 2 * D],
                        st3[:, w * WIN : (w + 1) * WIN, 0:D],
                    )
                elif w < VCOPY:
                    # ACT stages v to fp16 SBUF so the DVE multiply runs
                    # all-2-byte (2x path); relieves the DVE backlog
                    vc = vcp.tile([128, WIN * D], R, tag="vc")
                    vc3 = vc[:].rearrange("p (c x) -> p c x", c=WIN)
                    nc.scalar.copy(vc3[:, :, :], pp3[:, :, D : 2 * D])
                    nc.vector.tensor_mul(
                        st3[:, w * WIN : (w + 1) * WIN, D : 2 * D],
                        vc3[:, :, :],
                        st3[:, w * WIN : (w + 1) * WIN, 0:D],
                    )
                else:
                    nc.vector.tensor_mul(
                        st3[:, w * WIN : (w + 1) * WIN, D : 2 * D],
                        pp3[:, :, D : 2 * D],
                        st3[:, w * WIN : (w + 1) * WIN, 0:D],
                    )
                if w == n_win - 1 and len(pending_seg) >= SEG_DELAY:
                    emit_seg(len(pending_seg) - SEG_DELAY + 1)

            # previous group's tree at end-of-group: R1 never delays the
            # psum-critical muls on the in-order DVE queue
            emit_tree(1)
            pending_tree.append(
                (st, blk, oht, t == 0, t == gpbs[blk] - 1)
            )

        while pending_tree or pending_seg:
            emit_tree(1)
            emit_seg(len(pending_seg))

    _split_waits(nc)
    return nc


def _pack_host(gid, m):
    """Assign blocks (of 128 graphs) to core slots and compute per-slot
    group counts."""
    G = BPC * GBLK * N_CORES
    n_blocks = G // GBLK
    counts = np.bincount(gid, minlength=G)
    order = np.argsort(gid, kind="stable")
    g_starts = np.concatenate([[0], np.cumsum(counts)])

    lanes_per_block = np.array(
        [
            int(np.ceil(counts[b * GBLK : (b + 1) * GBLK] / m).sum())
            for b in range(n_blocks)
        ]
    )
    # slot assignment: sort blocks desc, slot k gets ranks k*8..k*8+7
    rank = np.argsort(-lanes_per_block)
    assign = np.zeros((N_CORES, BPC), np.int64)
    gpbs = []
    for k in range(BPC):
        blks = rank[k * N_CORES : (k + 1) * N_CORES]
        for c in range(N_CORES):
            assign[c, k] = blks[c]
        gpbs.append(int(np.ceil(lanes_per_block[blks].max() / 128)))
    return assign, tuple(gpbs), counts, order, g_starts


def kernel(
    f_node,
    key_W,
    key_b,
    value_W,
    value_b,
    gamma,
    beta,
    graph_id,
    num_graphs,
    trace=False,
):
    global LAST_EXEC_TIME_NS, LAST_RESULTS
    f_node = np.asarray(f_node, dtype=np.float32)
    key_W = np.asarray(key_W, dtype=np.float32)
    key_b = np.asarray(key_b, dtype=np.float32)
    value_W = np.asarray(value_W, dtype=np.float32)
    value_b = np.asarray(value_b, dtype=np.float32)
    gamma = np.asarray(gamma, dtype=np.float32)
    beta = np.asarray(beta, dtype=np.float32)
    gid = np.asarray(graph_id).astype(np.int64)
    G = int(num_graphs)
    m = PACK

    L, d = f_node.shape
    assert d == D and G == BPC * GBLK * N_CORES

    assign, gpbs, counts, order, g_starts = _pack_host(gid, m)
    TG = sum(gpbs)

    # f extended with the pad row: attn(f_pad) == PAD_ATT in every column
    f_pad = np.linalg.solve(
        key_W.astype(np.float64),
        np.full(D, PAD_ATT, np.float64) - key_b.astype(np.float64),
    ).astype(np.float32)
    f_ext = np.concatenate([f_node, f_pad[None, :]], axis=0)
    PADIDX = L

    wcat = np.ascontiguousarray(
        np.concatenate([key_W.T, value_W.T], axis=1)
    ).reshape(2, 128, 2 * D)
    vb_rep = np.ascontiguousarray(np.broadcast_to(value_b, (128, D)))
    eps_rep = np.ascontiguousarray(
        np.broadcast_to(
            (EPS_SOFTMAX / np.exp(key_b)).astype(np.float32), (128, D)
        )
    )
    gm_rep = np.ascontiguousarray(np.broadcast_to(gamma, (128, D)))
    bt_rep = np.ascontiguousarray(np.broadcast_to(beta, (128, D)))
    wcat16 = wcat.astype(np.float16)

    in_maps = []
    ymap = []  # (core, slot) -> block id
    for c in range(N_CORES):
        idx = np.full((TG, m, 128), PADIDX, np.int64)  # [group, chunk, lane]
        ohm = np.zeros((TG, 128, GBLK), np.float16)
        tg0 = 0
        for k in range(BPC):
            b = assign[c, k]
            lane = 0  # lane index within this block's group range
            for gl in range(GBLK):
                g = b * GBLK + gl
                n = counts[g]
                s = g_starts[g]
                nodes = order[s : s + n]
                for ls in range(0, n, m):
                    t = tg0 + lane // 128
                    li = lane % 128
                    seg = nodes[ls : ls + m]
                    idx[t, 0 : len(seg), li] = seg
                    ohm[t, li, gl] = 1.0
                    lane += 1
            tg0 += gpbs[k]
        # fT: [128, TG*2*GW]; col = t*(2*GW) + h*GW + j*128 + lane
        cols = idx.reshape(-1)  # [TG*m*128] node ids, (t, j, lane) order
        fshard = f_ext[cols].astype(np.float16)  # [ncols, D]
        fT = np.ascontiguousarray(
            fshard.reshape(TG, m, 128, 2, 128).transpose(4, 0, 3, 1, 2)
        ).reshape(128, TG * 2 * m * 128)
        ohT = np.ascontiguousarray(ohm.transpose(1, 0, 2)).reshape(
            128, TG * GBLK
        )
        in_maps.append(
            {
                "fT": fT,
                "oh": ohT,
                "wcat": wcat16,
                "vbrep": vb_rep,
                "epsrep": eps_rep,
                "gammarep": gm_rep,
                "betarep": bt_rep,
            }
        )
        ymap.append([assign[c, k] for k in range(BPC)])

    key = (gpbs, m)
    if key not in _nc_cache:
        _nc_cache[key] = _build_nc3(gpbs, m)
    nc = _nc_cache[key]

    if trace:
        _install_ntff_hook()
    res = run_bass_kernel_spmd(
        nc, in_maps, core_ids=list(range(N_CORES)), trace=trace
    )
    LAST_EXEC_TIME_NS = res.exec_time_ns
    LAST_RESULTS = res

    out = np.zeros((G, D), np.float32)
    for c in range(N_CORES):
        yc = res.results[c]["y"]
        for k in range(BPC):
            b = ymap[c][k]
            out[b * GBLK : (b + 1) * GBLK] = yc[k * GBLK : (k + 1) * GBLK]
    return out


def _install_ntff_hook():
    import sys, types

    try:
        if "antenv.axon_hooks" in sys.modules:
            return
        mod = types.ModuleType("antenv.axon_hooks")
        state = {"hook": None}
        mod.set_axon_ntff_profile_hook = lambda h: state.__setitem__("hook", h)
        mod.get_axon_ntff_profile_hook = lambda: state["hook"]
        sys.modules["antenv.axon_hooks"] = mod
        import antenv

        antenv.axon_hooks = mod
        from trn_agent_boot.trn_boot import _ntff_profile_via_ctypes

        mod.set_axon_ntff_profile_hook(
            _ntff_profile_via_ctypes("/opt/axon/libaxon_pjrt.so")
        )
    except Exception:
        pass
